# revision 54
# baseline (speedup 1.0000x reference)
"""TRN2 Bass kernel for nn_DiffQuantumSimulator (QAOA MaxCut, 18 qubits, p=4).

Strategy: data-parallel over batch (8 graphs -> 8 NeuronCores). Per core the
2^18 statevector lives in SBUF as [128 partitions x 2048] (re/im fp16 split,
one fused tile s2 = [s_re | s_im]).

Each QAOA layer applies exp(-i*hp) (diagonal) and the mixer RX(beta)^(x)18.
The mixer runs in 3 TensorE matmul phases:
  A: 128x128 complex gate RX^(x)7 on the 7 partition bits, fused with a
     partition<->free-bit transpose by using the *state* as the stationary
     operand (out = state_tile^T @ [C|D]).
  B: same trick on the next 7 bits.
  C: standard matmul applying RX^(x)4 (x) I_8 to the remaining 4 bits.
All matmuls run in float16 (full PE rate, half-cost LDWEIGHTS vs fp32r).

Optimization notes (measured on HW: elementwise passes cost a flat
~1.33 ns/elem on both DVE and Activation regardless of dtype/space --
the advertised DVE 2x/4x packed modes do NOT engage -- so the game is
pass count, engine balance and keeping the PE stream dense; the PE
ramps to full clock only after ~3us of continuous work and is
duty-cycle throttled (HAM) under sustained full-rate activity):
 - A's PSUM groups drain in ONE Activation pass per group with a
   3-free-dim de-interleaving dst AP (re/im to the two halves of t1).
 - B drains keep t2's interleaved layout: one contiguous pass per group.
 - B and C matmuls are interleaved (B0 B1 C0 B2 C1 B3 C2 C3) so phase-C
   chunks (and the rotations they feed) start mid-phase-B, which spreads
   the rotation work over a ~2x longer window and relieves PSUM pressure
   (C chunk k only needs B group k's drain).
 - KEY RESTRUCTURE: the inter-layer rotation does ONLY the 4 multiplies
   (DVE STT reading phase-C PSUM directly; the subtract sign folds into
   the STT scalar slot). The two complex-combine ADDS are performed for
   free by the NEXT layer's phase-A PSUM accumulation: phase A takes the
   four uncombined products (mRe0+mRe1 = s_re, mIm0+mIm1 = s_im) as four
   accumulating stationaries per window. This doubles phase A's moving
   columns but removes the combine latency/throughput from the DVE and
   gpsimd entirely, which previously serialized the whole C->A
   transition (measured: layer period 19.7us -> 16.8us).
 - The energy sum uses sqrt(hp) precomputed on host: e = sum((re*rt)^2 +
   (im*rt)^2): 3 DVE passes per chunk with the reduction in the DVE
   accumulator (no activation table needed anywhere); the final chunk is
   split into two PSUM halves (pre/pim in separate banks!) to halve the
   closing serial chain, and each partial column streams out over its
   own small DMA as soon as it is ready.
 - All input DMAs issue from the otherwise-idle Sync queue (+gpsimd);
   the first A-group's operands lead the queues so the PE starts ASAP.
 - FINE-GRAINED PSUM RELEASE (worth ~6us): every phase's PSUM lives in
   single-bank tiles (separate pre/pim tiles for C; two half-tiles with
   per-half drains for each A/B group), because the tile framework's
   WAR tracking is per-tile -- a shared tile makes downstream matmuls
   wait for ALL of the tile's readers (drains/rotations), serializing
   the PE stream. With per-bank tiles each bank frees the moment its
   own consumer finishes. Same reason the L3 energy square-accumulates
   are deferred behind all the PSUM-freeing tt multiplies.
"""

import numpy as np

import concourse.bass as bass
import concourse.mybir as mybir
import concourse.tile as tile
from concourse import bacc
from concourse.bass_utils import run_bass_kernel_spmd

N = 18
DIM = 1 << N
P = 128
F = DIM // P  # 2048
LAYERS = 4
BATCH = 8
NCORES = 8

FP32 = mybir.dt.float32
FP16 = mybir.dt.float16
ALU = mybir.AluOpType
ACT = mybir.ActivationFunctionType

# ----------------------------------------------------------------------------
# Host-side math: hp diagonal, gate matrices, bit-layout permutations
# ----------------------------------------------------------------------------


def _compute_hp(adj):
    W = (np.triu(adj, k=1) > 0.5).astype(np.float64)
    n_edges = W.sum()
    idx = np.arange(DIM)
    shifts = (N - 1 - np.arange(N))[:, None]
    Z = 1.0 - 2.0 * ((idx[None, :] >> shifts) & 1).astype(np.float64)
    T = W @ Z
    cross = np.einsum("ud,ud->d", T, Z)
    return 0.5 * (n_edges - cross)  # [DIM], integer-valued*0.5, exact


def _rx(beta):
    c, s = np.cos(beta), np.sin(beta)
    return np.array([[c, -1j * s], [-1j * s, c]], dtype=np.complex128)


def _kron_list(mats):
    out = np.array([[1.0]], dtype=np.complex128)
    for m in mats:
        out = np.kron(out, m)
    return out


def _m7(beta):
    return _kron_list([_rx(beta)] * 7)


def _m41(beta):
    return _kron_list([_rx(beta)] * 4 + [np.eye(2, dtype=np.complex128)] * 3)


def _bitmap_after_A(bm):
    new = [0] * N
    for j in range(7):
        new[11 + j] = bm[j]
    for j in range(4):
        new[7 + j] = bm[7 + j]
    for j in range(7):
        new[j] = bm[11 + j]
    return new


def _bitmap_after_B(bm):
    # window = free bits 10..4 (single strided AP dim), tiles = bits 3..0
    new = [0] * N
    for j in range(7):
        new[11 + j] = bm[4 + j]
    for j in range(4):
        new[7 + j] = bm[j]
    for j in range(7):
        new[j] = bm[11 + j]
    return new


def _perm_for_bitmap(bm):
    a = np.arange(DIM, dtype=np.int64)
    out = np.zeros(DIM, dtype=np.int64)
    for j in range(N):
        out |= ((a >> j) & 1) << bm[j]
    return out


def _layer_perms():
    """Permutations (orig_idx = perm[cur_idx]) for the state layout at the
    start of each layer (1..LAYERS) plus the final layout (index LAYERS)."""
    perms = []
    bm = list(range(N))
    for _ in range(LAYERS):
        perms.append(_perm_for_bitmap(bm))
        bm = _bitmap_after_B(_bitmap_after_A(bm))
    perms.append(_perm_for_bitmap(bm))
    return perms


_PERMS = _layer_perms()


def _host_prep(batch_betas, adj_matrices):
    """Build per-core input dicts."""
    in_maps = []
    for b in range(BATCH):
        hp = _compute_hp(np.asarray(adj_matrices[b], dtype=np.float64))
        cos_hp = np.cos(hp)
        sin_hp = np.sin(hp)

        init_re = (
            cos_hp[_PERMS[0]].astype(np.float16).reshape(P, 4, 512)
            .transpose(1, 0, 2).copy()
        )
        init_im = (
            (-sin_hp[_PERMS[0]]).astype(np.float16).reshape(P, 4, 512)
            .transpose(1, 0, 2).copy()
        )

        diags = np.empty((2 * (LAYERS - 1) + 1, P, F), dtype=np.float16)
        for t in range(1, LAYERS):
            diags[2 * (t - 1)] = cos_hp[_PERMS[t]].astype(np.float16).reshape(P, F)
            diags[2 * (t - 1) + 1] = sin_hp[_PERMS[t]].astype(np.float16).reshape(P, F)
        # sqrt(hp): hp is a nonnegative multiple of 0.5, so this is exact
        # enough in fp16 and lets the energy be sum((s*rt)^2).
        diags[-1] = np.sqrt(hp)[_PERMS[LAYERS]].astype(np.float16).reshape(P, F)

        gates_ab = np.empty((LAYERS, P, 512), dtype=np.float16)
        gates_c = np.empty((LAYERS, P, 384), dtype=np.float16)
        for t in range(LAYERS):
            beta = float(np.asarray(batch_betas[b][t], dtype=np.float64))
            M7 = _m7(beta)
            C7 = M7.real.astype(np.float16)
            D7 = M7.imag.astype(np.float16)
            M41 = _m41(beta)
            C41 = M41.real.astype(np.float16)
            D41 = M41.imag.astype(np.float16)
            gates_ab[t, :, 0:128] = C7
            gates_ab[t, :, 128:256] = D7
            gates_ab[t, :, 256:384] = -D7
            gates_ab[t, :, 384:512] = C7
            gates_c[t, :, 0:128] = C41
            gates_c[t, :, 128:256] = -D41
            gates_c[t, :, 256:384] = D41

        in_maps.append(
            {
                "init_re": init_re,
                "init_im": init_im,
                "diags": diags,
                "gates_ab": gates_ab,
                "gates_c": gates_c,
            }
        )
    return in_maps


# ----------------------------------------------------------------------------
# Bass program
# ----------------------------------------------------------------------------


def _build_program():
    nc = bacc.Bacc("TRN2", target_bir_lowering=False, debug=False)

    d_init_re = nc.dram_tensor("init_re", [4, P, 512], FP16, kind="ExternalInput")
    d_init_im = nc.dram_tensor("init_im", [4, P, 512], FP16, kind="ExternalInput")
    d_diags = nc.dram_tensor(
        "diags", [2 * (LAYERS - 1) + 1, P, F], FP16, kind="ExternalInput"
    )
    d_gates_ab = nc.dram_tensor("gates_ab", [LAYERS, P, 512], FP16, kind="ExternalInput")
    d_gates_c = nc.dram_tensor("gates_c", [LAYERS, P, 384], FP16, kind="ExternalInput")
    d_out = nc.dram_tensor("out", [P, 8], FP32, kind="ExternalOutput")

    n_diag = 2 * (LAYERS - 1) + 1

    def stt(eng, out, in0, in1, op, scalar=1.0, accum_out=None):
        # (in0 * scalar) op in1 -- plain fp16 tensor_tensor is ~2x slower
        eng.scalar_tensor_tensor(out, in0, scalar, in1, ALU.mult, op,
                                 accum_out=accum_out)

    with tile.TileContext(nc) as tc:
        with (
            tc.tile_pool(name="state", bufs=1) as st_pool,
            tc.tile_pool(name="consts", bufs=1) as c_pool,
            tc.tile_pool(name="rot", bufs=3) as h_pool,
            tc.tile_pool(name="ps_mm", bufs=2, space="PSUM") as ps_pool,
            tc.tile_pool(name="ps_c", bufs=2, space="PSUM") as ps_c,
        ):
            # fused state tile: s_re = s2[:, 0:F], s_im = s2[:, F:2F]
            # (used only for the DMA'd initial state of layer 0)
            s2 = st_pool.tile([P, 2 * F], FP16, tag="s2")
            # Uncombined rotation products for layers 1..3: phase A's PSUM
            # accumulation performs the complex-combine adds for free:
            #   s_re = mRe0 + mRe1,  s_im = mIm0 + mIm1
            # so the rotation is only 4 DVE multiplies per chunk, no adds.
            mRe0 = st_pool.tile([P, F], FP16, tag="mRe0")
            mRe1 = st_pool.tile([P, F], FP16, tag="mRe1")
            mIm0 = st_pool.tile([P, F], FP16, tag="mIm0")
            mIm1 = st_pool.tile([P, F], FP16, tag="mIm1")
            # fused t1 tile: t1_re = [:, 0:F], t1_im = [:, F:2F]
            t1 = st_pool.tile([P, 2 * F], FP16, tag="t1")
            # B output keeps PSUM's interleaved layout (per 128-col window:
            # re|im) so each B drain is one contiguous copy; phase C reads
            # it through a strided moving AP.
            t2 = st_pool.tile([P, 2 * F], FP16, tag="t2")

            diag_t = [
                c_pool.tile([P, F], FP16, tag=f"diag{k}", name=f"diag{k}")
                for k in range(n_diag)
            ]
            gates_ab_t = [
                c_pool.tile([P, 512], FP16, tag=f"gab{t}", name=f"gab{t}")
                for t in range(LAYERS)
            ]
            gates_c_t = [
                c_pool.tile([P, 384], FP16, tag=f"gc{t}", name=f"gc{t}")
                for t in range(LAYERS)
            ]
            partial = c_pool.tile([P, 8], FP32, tag="partial")

            s_re = s2[:, 0:F]
            s_im = s2[:, F : 2 * F]
            # de-interleaved t1 views
            t1_re = t1[:, 0:F]
            t1_im = t1[:, F : 2 * F]

            # ---- PE warm-up: the PE reaches max clock only after ~3us of
            # continuous activity; run dummy matmuls on a zeroed scratch
            # tile during the input-DMA wait so the real stream starts at
            # full rate. Their PSUM tile is never read (warnings are fine).
            warm = c_pool.tile([P, 256], FP16, tag="warm")
            nc.vector.memset(warm[:], 0.0)
            wps = ps_pool.tile([P, 512], FP32, tag="ps0", name="warmps")
            for i in range(12):
                nc.tensor.matmul(wps[:, 0:256], warm[:, 0:128], warm[:],
                                 start=True, stop=True)

            # ---- input DMAs: the three operands of the first A-group lead
            # three different queues so the first matmul starts ASAP.
            nc.sync.dma_start(gates_ab_t[0][:], d_gates_ab.ap()[0])
            nc.gpsimd.dma_start(s_re[:, 0:512], d_init_re.ap()[0])
            nc.sync.dma_start(s_im[:, 0:512], d_init_im.ap()[0])
            for c in range(1, 4):
                q = nc.sync if c % 2 == 1 else nc.gpsimd
                q.dma_start(s_re[:, 512 * c : 512 * (c + 1)], d_init_re.ap()[c])
                q2 = nc.gpsimd if c % 2 == 1 else nc.sync
                q2.dma_start(s_im[:, 512 * c : 512 * (c + 1)], d_init_im.ap()[c])
            nc.sync.dma_start(gates_c_t[0][:], d_gates_c.ap()[0])
            for t in range(1, LAYERS):
                nc.sync.dma_start(gates_ab_t[t][:], d_gates_ab.ap()[t])
                nc.sync.dma_start(gates_c_t[t][:], d_gates_c.ap()[t])

            def issue_diag_dmas(t):
                if t < LAYERS - 1:
                    nc.sync.dma_start(diag_t[2 * t][:], d_diags.ap()[2 * t])
                    nc.sync.dma_start(diag_t[2 * t + 1][:], d_diags.ap()[2 * t + 1])
                if t == LAYERS - 2:
                    # sqrt(hp) diagonal for the final energy reduction
                    nc.sync.dma_start(diag_t[n_diag - 1][:], d_diags.ap()[n_diag - 1])

            # strided views: B stationary windows / C moving chunks
            t1r4 = t1_re.rearrange("p (w u) -> p w u", w=128)
            t1i4 = t1_im.rearrange("p (w u) -> p w u", w=128)
            t2v = t2[:].rearrange("p (u ri j) -> p u ri j", u=16, ri=2)

            for t in range(LAYERS):
                cd7 = gates_ab_t[t][:, 0:256]
                ndc7 = gates_ab_t[t][:, 256:512]
                c41 = gates_c_t[t][:, 0:128]
                nd41 = gates_c_t[t][:, 128:256]
                d41 = gates_c_t[t][:, 256:384]

                # ---- phase A: stationary = state (contiguous 128-col windows).
                # Layer 0 reads the DMA'd state (2 matmuls/window); layers
                # 1..3 read the 4 uncombined rotation products (4 matmuls/
                # window) -- PSUM accumulation does the complex adds.
                dstv = t1[:].rearrange("p (ri g2 j T) -> p g2 j ri T", ri=2, g2=4, j=4)
                for g in range(4):
                    # two single-bank PSUM tiles per group: each half drains
                    # (and its bank frees) right after its own 2 windows
                    ph = [
                        ps_pool.tile([P, 512], FP32, tag=f"ps{h}", name=f"psA{t}{g}{h}")
                        for h in range(2)
                    ]
                    for j in range(4):
                        w = 4 * g + j
                        sl = slice(128 * w, 128 * (w + 1))
                        out_sl = ph[j // 2][:, 256 * (j % 2) : 256 * (j % 2 + 1)]
                        if t == 0:
                            nc.tensor.matmul(out_sl, s_re[:, sl], cd7, start=True, stop=False)
                            nc.tensor.matmul(out_sl, s_im[:, sl], ndc7, start=False, stop=True)
                        else:
                            # rotation chunks stay uncombined; PSUM adds them
                            nc.tensor.matmul(out_sl, mRe0[:, sl], cd7, start=True, stop=False)
                            nc.tensor.matmul(out_sl, mRe1[:, sl], cd7, start=False, stop=False)
                            nc.tensor.matmul(out_sl, mIm0[:, sl], ndc7, start=False, stop=False)
                            nc.tensor.matmul(out_sl, mIm1[:, sl], ndc7, start=False, stop=True)
                        if j % 2 == 1:
                            # de-interleaving half-drain: PSUM [j2; ri; T]
                            # scatters into the two halves of t1. The very
                            # last half (gating phase B) goes to the idle
                            # DVE so it runs parallel to Act's previous one.
                            h = j // 2
                            src = ph[h][:].rearrange("p (j2 ri T) -> p j2 ri T", j2=2, ri=2)
                            if g == 3 and h == 1:
                                nc.vector.tensor_copy(dstv[:, g, 2:4], src)
                            else:
                                nc.scalar.copy(dstv[:, g, 2 * h : 2 * h + 2], src)
                    if g == 0:
                        issue_diag_dmas(t)

                # ---- phases B and C, interleaved: B0 B1 C0 B2 C1 B3 C2 C3.
                # C chunk k only needs B group k's drain, so phase C (and the
                # rotation it feeds) starts mid-B, spreading the DVE rotation
                # work over a ~2x longer window and relieving PSUM pressure.
                def b_group(g):
                    ph = [
                        ps_pool.tile([P, 512], FP32, tag=f"ps{h}", name=f"psB{t}{g}{h}")
                        for h in range(2)
                    ]
                    for j in range(4):
                        u = 4 * g + j
                        out_sl = ph[j // 2][:, 256 * (j % 2) : 256 * (j % 2 + 1)]
                        nc.tensor.matmul(
                            out_sl, t1r4[:, :, u], cd7, start=True, stop=False
                        )
                        nc.tensor.matmul(
                            out_sl, t1i4[:, :, u], ndc7, start=False, stop=True
                        )
                        if j % 2 == 1:
                            h = j // 2
                            dst = slice(1024 * g + 512 * h, 1024 * g + 512 * (h + 1))
                            nc.scalar.copy(t2[:, dst], ph[h][:])

                def c_chunk(k):
                    if t == LAYERS - 1 and k == 3:
                        # final chunk: two independent PSUM half-chunks with
                        # their OWN tiles (sharing a tile would make half-b's
                        # matmuls wait on half-a's energy reads) so the
                        # closing energy chain is half as long
                        rt = diag_t[n_diag - 1]
                        for h in range(2):
                            pcr = ps_c.tile([P, 512], FP32, tag="pcr", name=f"pcr3{h}")
                            pci = ps_c.tile([P, 512], FP32, tag="pci", name=f"pci3{h}")
                            mv_re = t2v[:, 4 * k + 2 * h : 4 * k + 2 * h + 2, 0, :]
                            mv_im = t2v[:, 4 * k + 2 * h : 4 * k + 2 * h + 2, 1, :]
                            pre = pcr[:, 0:256]
                            pim = pci[:, 0:256]
                            nc.tensor.matmul(pre, c41, mv_re, start=True, stop=False)
                            nc.tensor.matmul(pim, c41, mv_im, start=True, stop=False)
                            nc.tensor.matmul(pre, nd41, mv_im, start=False, stop=True)
                            nc.tensor.matmul(pim, d41, mv_re, start=False, stop=True)
                            hk = slice(512 * k + 256 * h, 512 * k + 256 * (h + 1))
                            tt = h_pool.tile([P, 512], FP16, tag="th", name=f"tt3{h}")
                            sq = h_pool.tile([P, 512], FP16, tag="sh", name=f"sq3{h}")
                            stt(nc.vector, tt[:, 0:256], pre, rt[:, hk], ALU.mult)
                            stt(nc.vector, tt[:, 256:512], pim, rt[:, hk], ALU.mult)
                            stt(nc.vector, sq[:], tt[:], tt[:], ALU.mult,
                                accum_out=partial[:, 3 + h : 4 + h])
                            nc.sync.dma_start(d_out.ap()[:, 3 + h : 4 + h],
                                              partial[:, 3 + h : 4 + h])
                        return
                    # separate single-bank tiles for pre/pim so each bank is
                    # released as soon as its own rotation reads finish
                    pcr = ps_c.tile([P, 512], FP32, tag="pcr", name=f"pcr{t}{k}")
                    pci = ps_c.tile([P, 512], FP32, tag="pci", name=f"pci{t}{k}")
                    mv_re = t2v[:, 4 * k : 4 * (k + 1), 0, :]
                    mv_im = t2v[:, 4 * k : 4 * (k + 1), 1, :]
                    pre = pcr[:]
                    pim = pci[:]
                    nc.tensor.matmul(pre, c41, mv_re, start=True, stop=False)
                    nc.tensor.matmul(pim, c41, mv_im, start=True, stop=False)
                    nc.tensor.matmul(pre, nd41, mv_im, start=False, stop=True)
                    nc.tensor.matmul(pim, d41, mv_re, start=False, stop=True)

                    ck = slice(512 * k, 512 * (k + 1))
                    if t < LAYERS - 1:
                        # state = psC * exp(-i hp):
                        #   re' = re*c + im*s ; im' = im*c - re*s
                        # Only the 4 multiplies run here (DVE, reading PSUM
                        # directly; pre-consumers ordered first so its bank
                        # frees early); the combine adds happen inside next
                        # layer's phase-A PSUM accumulation via the pieces.
                        cs = diag_t[2 * t][:, ck]
                        sn = diag_t[2 * t + 1][:, ck]
                        stt(nc.vector, mRe0[:, ck], pre, cs, ALU.mult)
                        stt(nc.vector, mIm1[:, ck], pre, sn, ALU.mult,
                            scalar=-1.0)
                        stt(nc.vector, mRe1[:, ck], pim, sn, ALU.mult)
                        stt(nc.vector, mIm0[:, ck], pim, cs, ALU.mult)
                    else:
                        # energy: sum(|state*rt|^2), rt = sqrt(hp).
                        # Only the PSUM-freeing tt multiplies run here; the
                        # square-accumulates are deferred so later C chunks
                        # never wait on PSUM banks held by the backlog.
                        rt = diag_t[n_diag - 1][:, ck]
                        tt = h_pool.tile([P, 1024], FP16, tag="mm", name=f"tt{k}")
                        stt(nc.vector, tt[:, 0:512], pre, rt, ALU.mult)
                        stt(nc.vector, tt[:, 512:1024], pim, rt, ALU.mult)
                        pending_sq.append((tt, k))

                def flush_sq():
                    for tt, k in pending_sq:
                        sq = h_pool.tile([P, 1024], FP16, tag="mn", name=f"sq{k}")
                        stt(nc.vector, sq[:], tt[:], tt[:], ALU.mult,
                            accum_out=partial[:, k : k + 1])
                        nc.sync.dma_start(d_out.ap()[:, k : k + 1],
                                          partial[:, k : k + 1])
                    pending_sq.clear()

                pending_sq = []
                b_group(0)
                b_group(1)
                c_chunk(0)
                b_group(2)
                c_chunk(1)
                b_group(3)
                c_chunk(2)
                flush_sq()
                c_chunk(3)

    nc.compile()
    return nc


_NC_CACHE = {}


def _get_program():
    if "nc" not in _NC_CACHE:
        _NC_CACHE["nc"] = _build_program()
    return _NC_CACHE["nc"]


def kernel(batch_betas, adj_matrices, _trace=False, _tmpdir=None):
    batch_betas = np.asarray(batch_betas, dtype=np.float32)
    adj_matrices = np.asarray(adj_matrices, dtype=np.float32)
    assert batch_betas.shape == (BATCH, LAYERS)
    assert adj_matrices.shape == (BATCH, N, N)

    nc = _get_program()
    in_maps = _host_prep(batch_betas, adj_matrices)
    res = run_bass_kernel_spmd(
        nc,
        in_maps,
        list(range(NCORES)),
        trace=_trace,
        tmpdir=_tmpdir,
    )
    energies = np.array(
        [res.results[b]["out"][:, 0:5].sum() / DIM for b in range(BATCH)],
        dtype=np.float32,
    )
    if _trace:
        return energies, res
    return energies


# revision 55
# speedup vs baseline: 1.1742x; 1.1742x over previous
"""TRN2 Bass kernel for nn_DiffQuantumSimulator (QAOA MaxCut, 18 qubits, p=4).

Strategy: data-parallel over batch (8 graphs -> 8 NeuronCores). Per core the
2^18 statevector lives in SBUF as [128 partitions x 2048] (re/im fp16 split,
one fused tile s2 = [s_re | s_im]).

Each QAOA layer applies exp(-i*hp) (diagonal) and the mixer RX(beta)^(x)18.
The mixer runs in 3 TensorE matmul phases:
  A: 128x128 complex gate RX^(x)7 on the 7 partition bits, fused with a
     partition<->free-bit transpose by using the *state* as the stationary
     operand (out = state_tile^T @ [C|D]).
  B: same trick on the next 7 bits.
  C: standard matmul applying RX^(x)4 (x) I_8 to the remaining 4 bits.
All matmuls run in float16 (full PE rate, half-cost LDWEIGHTS vs fp32r).

Optimization notes (measured on HW: elementwise passes cost a flat
~1.33 ns/elem on both DVE and Activation regardless of dtype/space --
the advertised DVE 2x/4x packed modes do NOT engage -- so the game is
pass count, engine balance and keeping the PE stream dense; the PE
ramps to full clock only after ~3us of continuous work and is
duty-cycle throttled (HAM) under sustained full-rate activity):
 - A's PSUM groups drain in ONE Activation pass per group with a
   3-free-dim de-interleaving dst AP (re/im to the two halves of t1).
 - B drains keep t2's interleaved layout: one contiguous pass per group.
 - B and C matmuls are interleaved (B0 B1 C0 B2 C1 B3 C2 C3) so phase-C
   chunks (and the rotations they feed) start mid-phase-B, which spreads
   the rotation work over a ~2x longer window and relieves PSUM pressure
   (C chunk k only needs B group k's drain).
 - KEY RESTRUCTURE: the inter-layer rotation does ONLY the 4 multiplies
   (DVE STT reading phase-C PSUM directly; the subtract sign folds into
   the STT scalar slot). The two complex-combine ADDS are performed for
   free by the NEXT layer's phase-A PSUM accumulation: phase A takes the
   four uncombined products (mRe0+mRe1 = s_re, mIm0+mIm1 = s_im) as four
   accumulating stationaries per window. This doubles phase A's moving
   columns but removes the combine latency/throughput from the DVE and
   gpsimd entirely, which previously serialized the whole C->A
   transition (measured: layer period 19.7us -> 16.8us).
 - The energy sum uses sqrt(hp) precomputed on host: e = sum((re*rt)^2 +
   (im*rt)^2): 3 DVE passes per chunk with the reduction in the DVE
   accumulator (no activation table needed anywhere); the final chunk is
   split into two PSUM halves (pre/pim in separate banks!) to halve the
   closing serial chain, and each partial column streams out over its
   own small DMA as soon as it is ready.
 - All input DMAs issue from the otherwise-idle Sync queue (+gpsimd);
   the first A-group's operands lead the queues so the PE starts ASAP.
 - FINE-GRAINED PSUM RELEASE (worth ~6us): every phase's PSUM lives in
   single-bank tiles (separate pre/pim tiles for C; two half-tiles with
   per-half drains for each A/B group), because the tile framework's
   WAR tracking is per-tile -- a shared tile makes downstream matmuls
   wait for ALL of the tile's readers (drains/rotations), serializing
   the PE stream. With per-bank tiles each bank frees the moment its
   own consumer finishes. Same reason the L3 energy square-accumulates
   are deferred behind all the PSUM-freeing tt multiplies.
"""

import numpy as np

import concourse.bass as bass
import concourse.mybir as mybir
import concourse.tile as tile
from concourse import bacc
from concourse.bass_utils import run_bass_kernel_spmd

N = 18
DIM = 1 << N
P = 128
F = DIM // P  # 2048
LAYERS = 4
BATCH = 8
NCORES = 8

FP32 = mybir.dt.float32
FP16 = mybir.dt.float16
ALU = mybir.AluOpType
ACT = mybir.ActivationFunctionType

# ----------------------------------------------------------------------------
# Host-side math: hp diagonal, gate matrices, bit-layout permutations
# ----------------------------------------------------------------------------


def _compute_hp(adj):
    W = (np.triu(adj, k=1) > 0.5).astype(np.float64)
    n_edges = W.sum()
    idx = np.arange(DIM)
    shifts = (N - 1 - np.arange(N))[:, None]
    Z = 1.0 - 2.0 * ((idx[None, :] >> shifts) & 1).astype(np.float64)
    T = W @ Z
    cross = np.einsum("ud,ud->d", T, Z)
    return 0.5 * (n_edges - cross)  # [DIM], integer-valued*0.5, exact


def _rx(beta):
    c, s = np.cos(beta), np.sin(beta)
    return np.array([[c, -1j * s], [-1j * s, c]], dtype=np.complex128)


def _kron_list(mats):
    out = np.array([[1.0]], dtype=np.complex128)
    for m in mats:
        out = np.kron(out, m)
    return out


def _m7(beta):
    return _kron_list([_rx(beta)] * 7)


def _m41(beta):
    return _kron_list([_rx(beta)] * 4 + [np.eye(2, dtype=np.complex128)] * 3)


def _bitmap_after_A(bm):
    new = [0] * N
    for j in range(7):
        new[11 + j] = bm[j]
    for j in range(4):
        new[7 + j] = bm[7 + j]
    for j in range(7):
        new[j] = bm[11 + j]
    return new


def _bitmap_after_B(bm):
    # window = free bits 10..4 (single strided AP dim), tiles = bits 3..0
    new = [0] * N
    for j in range(7):
        new[11 + j] = bm[4 + j]
    for j in range(4):
        new[7 + j] = bm[j]
    for j in range(7):
        new[j] = bm[11 + j]
    return new


def _perm_for_bitmap(bm):
    a = np.arange(DIM, dtype=np.int64)
    out = np.zeros(DIM, dtype=np.int64)
    for j in range(N):
        out |= ((a >> j) & 1) << bm[j]
    return out


def _layer_perms():
    """Permutations (orig_idx = perm[cur_idx]) for the state layout at the
    start of each layer (1..LAYERS) plus the final layout (index LAYERS)."""
    perms = []
    bm = list(range(N))
    for _ in range(LAYERS):
        perms.append(_perm_for_bitmap(bm))
        bm = _bitmap_after_B(_bitmap_after_A(bm))
    perms.append(_perm_for_bitmap(bm))
    return perms


_PERMS = _layer_perms()


def _host_prep(batch_betas, adj_matrices):
    """Build per-core input dicts."""
    in_maps = []
    for b in range(BATCH):
        hp = _compute_hp(np.asarray(adj_matrices[b], dtype=np.float64))
        cos_hp = np.cos(hp)
        sin_hp = np.sin(hp)

        init_re = (
            cos_hp[_PERMS[0]].astype(np.float16).reshape(P, 4, 512)
            .transpose(1, 0, 2).copy()
        )
        init_im = (
            (-sin_hp[_PERMS[0]]).astype(np.float16).reshape(P, 4, 512)
            .transpose(1, 0, 2).copy()
        )

        diags = np.empty((2 * (LAYERS - 1) + 1, P, F), dtype=np.float16)
        for t in range(1, LAYERS):
            diags[2 * (t - 1)] = cos_hp[_PERMS[t]].astype(np.float16).reshape(P, F)
            diags[2 * (t - 1) + 1] = sin_hp[_PERMS[t]].astype(np.float16).reshape(P, F)
        # sqrt(hp): hp is a nonnegative multiple of 0.5, so this is exact
        # enough in fp16 and lets the energy be sum((s*rt)^2).
        diags[-1] = np.sqrt(hp)[_PERMS[LAYERS]].astype(np.float16).reshape(P, F)

        gates_ab = np.empty((LAYERS, P, 512), dtype=np.float16)
        gates_c = np.empty((LAYERS, P, 384), dtype=np.float16)
        for t in range(LAYERS):
            beta = float(np.asarray(batch_betas[b][t], dtype=np.float64))
            M7 = _m7(beta)
            C7 = M7.real.astype(np.float16)
            D7 = M7.imag.astype(np.float16)
            M41 = _m41(beta)
            C41 = M41.real.astype(np.float16)
            D41 = M41.imag.astype(np.float16)
            gates_ab[t, :, 0:128] = C7
            gates_ab[t, :, 128:256] = D7
            gates_ab[t, :, 256:384] = -D7
            gates_ab[t, :, 384:512] = C7
            gates_c[t, :, 0:128] = C41
            gates_c[t, :, 128:256] = -D41
            gates_c[t, :, 256:384] = D41

        in_maps.append(
            {
                "init_re": init_re,
                "init_im": init_im,
                "diags": diags,
                "gates_ab": gates_ab,
                "gates_c": gates_c,
            }
        )
    return in_maps


# ----------------------------------------------------------------------------
# Bass program
# ----------------------------------------------------------------------------


def _build_program():
    nc = bacc.Bacc("TRN2", target_bir_lowering=False, debug=False)

    d_init_re = nc.dram_tensor("init_re", [4, P, 512], FP16, kind="ExternalInput")
    d_init_im = nc.dram_tensor("init_im", [4, P, 512], FP16, kind="ExternalInput")
    d_diags = nc.dram_tensor(
        "diags", [2 * (LAYERS - 1) + 1, P, F], FP16, kind="ExternalInput"
    )
    d_gates_ab = nc.dram_tensor("gates_ab", [LAYERS, P, 512], FP16, kind="ExternalInput")
    d_gates_c = nc.dram_tensor("gates_c", [LAYERS, P, 384], FP16, kind="ExternalInput")
    d_out = nc.dram_tensor("out", [P, 8], FP32, kind="ExternalOutput")

    n_diag = 2 * (LAYERS - 1) + 1

    def stt(eng, out, in0, in1, op, scalar=1.0, accum_out=None):
        # (in0 * scalar) op in1 -- plain fp16 tensor_tensor is ~2x slower
        eng.scalar_tensor_tensor(out, in0, scalar, in1, ALU.mult, op,
                                 accum_out=accum_out)

    with tile.TileContext(nc) as tc:
        with (
            tc.tile_pool(name="state", bufs=1) as st_pool,
            tc.tile_pool(name="consts", bufs=1) as c_pool,
            tc.tile_pool(name="rot", bufs=3) as h_pool,
            tc.tile_pool(name="ps_mm", bufs=2, space="PSUM") as ps_pool,
            tc.tile_pool(name="ps_c", bufs=2, space="PSUM") as ps_c,
        ):
            # fused state tile: s_re = s2[:, 0:F], s_im = s2[:, F:2F]
            # (used only for the DMA'd initial state of layer 0)
            s2 = st_pool.tile([P, 2 * F], FP16, tag="s2")
            # Uncombined rotation products for layers 1..3: phase A's PSUM
            # accumulation performs the complex-combine adds for free:
            #   s_re = mRe0 + mRe1,  s_im = mIm0 + mIm1
            # so the rotation is only 4 DVE multiplies per chunk, no adds.
            mRe0 = st_pool.tile([P, F], FP16, tag="mRe0")
            mRe1 = st_pool.tile([P, F], FP16, tag="mRe1")
            mIm0 = st_pool.tile([P, F], FP16, tag="mIm0")
            mIm1 = st_pool.tile([P, F], FP16, tag="mIm1")
            # fused t1 tile: t1_re = [:, 0:F], t1_im = [:, F:2F]
            t1 = st_pool.tile([P, 2 * F], FP16, tag="t1")
            # B output keeps PSUM's interleaved layout (per 128-col window:
            # re|im) so each B drain is one contiguous copy; phase C reads
            # it through a strided moving AP.
            t2 = st_pool.tile([P, 2 * F], FP16, tag="t2")

            diag_t = [
                c_pool.tile([P, F], FP16, tag=f"diag{k}", name=f"diag{k}")
                for k in range(n_diag)
            ]
            gates_ab_t = [
                c_pool.tile([P, 512], FP16, tag=f"gab{t}", name=f"gab{t}")
                for t in range(LAYERS)
            ]
            gates_c_t = [
                c_pool.tile([P, 384], FP16, tag=f"gc{t}", name=f"gc{t}")
                for t in range(LAYERS)
            ]
            partial = c_pool.tile([P, 8], FP32, tag="partial")

            s_re = s2[:, 0:F]
            s_im = s2[:, F : 2 * F]
            # de-interleaved t1 views
            t1_re = t1[:, 0:F]
            t1_im = t1[:, F : 2 * F]

            # ---- input DMAs: the three operands of the first A-group lead
            # three different queues so the first matmul starts ASAP.
            nc.sync.dma_start(gates_ab_t[0][:], d_gates_ab.ap()[0])
            nc.gpsimd.dma_start(s_re[:, 0:512], d_init_re.ap()[0])
            nc.sync.dma_start(s_im[:, 0:512], d_init_im.ap()[0])
            for c in range(1, 4):
                q = nc.sync if c % 2 == 1 else nc.gpsimd
                q.dma_start(s_re[:, 512 * c : 512 * (c + 1)], d_init_re.ap()[c])
                q2 = nc.gpsimd if c % 2 == 1 else nc.sync
                q2.dma_start(s_im[:, 512 * c : 512 * (c + 1)], d_init_im.ap()[c])
            nc.sync.dma_start(gates_c_t[0][:], d_gates_c.ap()[0])
            for t in range(1, LAYERS):
                nc.sync.dma_start(gates_ab_t[t][:], d_gates_ab.ap()[t])
                nc.sync.dma_start(gates_c_t[t][:], d_gates_c.ap()[t])

            def issue_diag_dmas(t):
                if t < LAYERS - 1:
                    nc.sync.dma_start(diag_t[2 * t][:], d_diags.ap()[2 * t])
                    nc.sync.dma_start(diag_t[2 * t + 1][:], d_diags.ap()[2 * t + 1])
                if t == LAYERS - 2:
                    # sqrt(hp) diagonal for the final energy reduction
                    nc.sync.dma_start(diag_t[n_diag - 1][:], d_diags.ap()[n_diag - 1])

            # strided views: B stationary windows / C moving chunks
            t1r4 = t1_re.rearrange("p (w u) -> p w u", w=128)
            t1i4 = t1_im.rearrange("p (w u) -> p w u", w=128)
            t2v = t2[:].rearrange("p (u ri j) -> p u ri j", u=16, ri=2)

            for t in range(LAYERS):
                cd7 = gates_ab_t[t][:, 0:256]
                ndc7 = gates_ab_t[t][:, 256:512]
                c41 = gates_c_t[t][:, 0:128]
                nd41 = gates_c_t[t][:, 128:256]
                d41 = gates_c_t[t][:, 256:384]

                # ---- phase A: stationary = state (contiguous 128-col windows).
                # Layer 0 reads the DMA'd state (2 matmuls/window); layers
                # 1..3 read the 4 uncombined rotation products (4 matmuls/
                # window) -- PSUM accumulation does the complex adds.
                dstv = t1[:].rearrange("p (ri g2 j T) -> p g2 j ri T", ri=2, g2=4, j=4)
                for g in range(4):
                    # two single-bank PSUM tiles per group: each half drains
                    # (and its bank frees) right after its own 2 windows
                    ph = [
                        ps_pool.tile([P, 512], FP32, tag=f"ps{h}", name=f"psA{t}{g}{h}")
                        for h in range(2)
                    ]
                    for j in range(4):
                        w = 4 * g + j
                        sl = slice(128 * w, 128 * (w + 1))
                        out_sl = ph[j // 2][:, 256 * (j % 2) : 256 * (j % 2 + 1)]
                        if t == 0:
                            nc.tensor.matmul(out_sl, s_re[:, sl], cd7, start=True, stop=False)
                            nc.tensor.matmul(out_sl, s_im[:, sl], ndc7, start=False, stop=True)
                        else:
                            # rotation chunks stay uncombined; PSUM adds them
                            nc.tensor.matmul(out_sl, mRe0[:, sl], cd7, start=True, stop=False)
                            nc.tensor.matmul(out_sl, mRe1[:, sl], cd7, start=False, stop=False)
                            nc.tensor.matmul(out_sl, mIm0[:, sl], ndc7, start=False, stop=False)
                            nc.tensor.matmul(out_sl, mIm1[:, sl], ndc7, start=False, stop=True)
                        if j % 2 == 1:
                            # de-interleaving half-drain: PSUM [j2; ri; T]
                            # scatters into the two halves of t1. The very
                            # last half (gating phase B) goes to the idle
                            # DVE so it runs parallel to Act's previous one.
                            h = j // 2
                            src = ph[h][:].rearrange("p (j2 ri T) -> p j2 ri T", j2=2, ri=2)
                            if g == 3 and h == 1:
                                nc.vector.tensor_copy(dstv[:, g, 2:4], src)
                            else:
                                nc.scalar.copy(dstv[:, g, 2 * h : 2 * h + 2], src)
                    if g == 0:
                        issue_diag_dmas(t)

                # ---- phases B and C, interleaved: B0 B1 C0 B2 C1 B3 C2 C3.
                # C chunk k only needs B group k's drain, so phase C (and the
                # rotation it feeds) starts mid-B, spreading the DVE rotation
                # work over a ~2x longer window and relieving PSUM pressure.
                def b_group(g):
                    ph = [
                        ps_pool.tile([P, 512], FP32, tag=f"ps{h}", name=f"psB{t}{g}{h}")
                        for h in range(2)
                    ]
                    for j in range(4):
                        u = 4 * g + j
                        out_sl = ph[j // 2][:, 256 * (j % 2) : 256 * (j % 2 + 1)]
                        nc.tensor.matmul(
                            out_sl, t1r4[:, :, u], cd7, start=True, stop=False
                        )
                        nc.tensor.matmul(
                            out_sl, t1i4[:, :, u], ndc7, start=False, stop=True
                        )
                        if j % 2 == 1:
                            h = j // 2
                            dst = slice(1024 * g + 512 * h, 1024 * g + 512 * (h + 1))
                            nc.scalar.copy(t2[:, dst], ph[h][:])

                def c_chunk(k):
                    if t == LAYERS - 1 and k == 3:
                        # final chunk: two independent PSUM half-chunks with
                        # their OWN tiles (sharing a tile would make half-b's
                        # matmuls wait on half-a's energy reads) so the
                        # closing energy chain is half as long
                        rt = diag_t[n_diag - 1]
                        for h in range(2):
                            pcr = ps_c.tile([P, 512], FP32, tag="pcr", name=f"pcr3{h}")
                            pci = ps_c.tile([P, 512], FP32, tag="pci", name=f"pci3{h}")
                            mv_re = t2v[:, 4 * k + 2 * h : 4 * k + 2 * h + 2, 0, :]
                            mv_im = t2v[:, 4 * k + 2 * h : 4 * k + 2 * h + 2, 1, :]
                            pre = pcr[:, 0:256]
                            pim = pci[:, 0:256]
                            nc.tensor.matmul(pre, c41, mv_re, start=True, stop=False)
                            nc.tensor.matmul(pim, c41, mv_im, start=True, stop=False)
                            nc.tensor.matmul(pre, nd41, mv_im, start=False, stop=True)
                            nc.tensor.matmul(pim, d41, mv_re, start=False, stop=True)
                            hk = slice(512 * k + 256 * h, 512 * k + 256 * (h + 1))
                            tt = h_pool.tile([P, 512], FP16, tag="th", name=f"tt3{h}")
                            sq = h_pool.tile([P, 512], FP16, tag="sh", name=f"sq3{h}")
                            stt(nc.vector, tt[:, 0:256], pre, rt[:, hk], ALU.mult)
                            stt(nc.vector, tt[:, 256:512], pim, rt[:, hk], ALU.mult)
                            stt(nc.vector, sq[:], tt[:], tt[:], ALU.mult,
                                accum_out=partial[:, 3 + h : 4 + h])
                            nc.sync.dma_start(d_out.ap()[:, 3 + h : 4 + h],
                                              partial[:, 3 + h : 4 + h])
                        return
                    # separate single-bank tiles for pre/pim so each bank is
                    # released as soon as its own rotation reads finish
                    pcr = ps_c.tile([P, 512], FP32, tag="pcr", name=f"pcr{t}{k}")
                    pci = ps_c.tile([P, 512], FP32, tag="pci", name=f"pci{t}{k}")
                    mv_re = t2v[:, 4 * k : 4 * (k + 1), 0, :]
                    mv_im = t2v[:, 4 * k : 4 * (k + 1), 1, :]
                    pre = pcr[:]
                    pim = pci[:]
                    nc.tensor.matmul(pre, c41, mv_re, start=True, stop=False)
                    nc.tensor.matmul(pim, c41, mv_im, start=True, stop=False)
                    nc.tensor.matmul(pre, nd41, mv_im, start=False, stop=True)
                    nc.tensor.matmul(pim, d41, mv_re, start=False, stop=True)

                    ck = slice(512 * k, 512 * (k + 1))
                    if t < LAYERS - 1:
                        # state = psC * exp(-i hp):
                        #   re' = re*c + im*s ; im' = im*c - re*s
                        # Only the 4 multiplies run here (DVE, reading PSUM
                        # directly; pre-consumers ordered first so its bank
                        # frees early); the combine adds happen inside next
                        # layer's phase-A PSUM accumulation via the pieces.
                        cs = diag_t[2 * t][:, ck]
                        sn = diag_t[2 * t + 1][:, ck]
                        stt(nc.vector, mRe0[:, ck], pre, cs, ALU.mult)
                        stt(nc.vector, mIm1[:, ck], pre, sn, ALU.mult,
                            scalar=-1.0)
                        stt(nc.vector, mRe1[:, ck], pim, sn, ALU.mult)
                        stt(nc.vector, mIm0[:, ck], pim, cs, ALU.mult)
                    else:
                        # energy: sum(|state*rt|^2), rt = sqrt(hp).
                        # Only the PSUM-freeing tt multiplies run here; the
                        # square-accumulates are deferred so later C chunks
                        # never wait on PSUM banks held by the backlog.
                        rt = diag_t[n_diag - 1][:, ck]
                        tt = h_pool.tile([P, 1024], FP16, tag="mm", name=f"tt{k}")
                        stt(nc.vector, tt[:, 0:512], pre, rt, ALU.mult)
                        stt(nc.vector, tt[:, 512:1024], pim, rt, ALU.mult)
                        pending_sq.append((tt, k))

                def flush_sq():
                    for tt, k in pending_sq:
                        sq = h_pool.tile([P, 1024], FP16, tag="mn", name=f"sq{k}")
                        stt(nc.vector, sq[:], tt[:], tt[:], ALU.mult,
                            accum_out=partial[:, k : k + 1])
                        nc.sync.dma_start(d_out.ap()[:, k : k + 1],
                                          partial[:, k : k + 1])
                    pending_sq.clear()

                pending_sq = []
                b_group(0)
                b_group(1)
                c_chunk(0)
                b_group(2)
                c_chunk(1)
                b_group(3)
                c_chunk(2)
                flush_sq()
                c_chunk(3)

    nc.compile()
    return nc


_NC_CACHE = {}


def _get_program():
    if "nc" not in _NC_CACHE:
        _NC_CACHE["nc"] = _build_program()
    return _NC_CACHE["nc"]


def kernel(batch_betas, adj_matrices, _trace=False, _tmpdir=None):
    batch_betas = np.asarray(batch_betas, dtype=np.float32)
    adj_matrices = np.asarray(adj_matrices, dtype=np.float32)
    assert batch_betas.shape == (BATCH, LAYERS)
    assert adj_matrices.shape == (BATCH, N, N)

    nc = _get_program()
    in_maps = _host_prep(batch_betas, adj_matrices)
    res = run_bass_kernel_spmd(
        nc,
        in_maps,
        list(range(NCORES)),
        trace=_trace,
        tmpdir=_tmpdir,
    )
    energies = np.array(
        [res.results[b]["out"][:, 0:5].sum() / DIM for b in range(BATCH)],
        dtype=np.float32,
    )
    if _trace:
        return energies, res
    return energies


# revision 56
# speedup vs baseline: 1.1957x; 1.0183x over previous
"""TRN2 Bass kernel for nn_DiffQuantumSimulator (QAOA MaxCut, 18 qubits, p=4).

Strategy: data-parallel over batch (8 graphs -> 8 NeuronCores). Per core the
2^18 statevector lives in SBUF as [128 partitions x 2048] (re/im fp16 split,
one fused tile s2 = [s_re | s_im]).

Each QAOA layer applies exp(-i*hp) (diagonal) and the mixer RX(beta)^(x)18.
The mixer runs in 3 TensorE matmul phases:
  A: 128x128 complex gate RX^(x)7 on the 7 partition bits, fused with a
     partition<->free-bit transpose by using the *state* as the stationary
     operand (out = state_tile^T @ [C|D]).
  B: same trick on the next 7 bits.
  C: standard matmul applying RX^(x)4 (x) I_8 to the remaining 4 bits.
All matmuls run in float16 (full PE rate, half-cost LDWEIGHTS vs fp32r).

Optimization notes (measured on HW: elementwise passes cost a flat
~1.33 ns/elem on both DVE and Activation regardless of dtype/space --
the advertised DVE 2x/4x packed modes do NOT engage -- so the game is
pass count, engine balance and keeping the PE stream dense; the PE
ramps to full clock only after ~3us of continuous work and is
duty-cycle throttled (HAM) under sustained full-rate activity):
 - A's PSUM groups drain in ONE Activation pass per group with a
   3-free-dim de-interleaving dst AP (re/im to the two halves of t1).
 - B drains keep t2's interleaved layout: one contiguous pass per group.
 - B and C matmuls are interleaved (B0 B1 C0 B2 C1 B3 C2 C3) so phase-C
   chunks (and the rotations they feed) start mid-phase-B, which spreads
   the rotation work over a ~2x longer window and relieves PSUM pressure
   (C chunk k only needs B group k's drain).
 - KEY RESTRUCTURE: the inter-layer rotation does ONLY the 4 multiplies
   (DVE STT reading phase-C PSUM directly; the subtract sign folds into
   the STT scalar slot). The two complex-combine ADDS are performed for
   free by the NEXT layer's phase-A PSUM accumulation: phase A takes the
   four uncombined products (mRe0+mRe1 = s_re, mIm0+mIm1 = s_im) as four
   accumulating stationaries per window. This doubles phase A's moving
   columns but removes the combine latency/throughput from the DVE and
   gpsimd entirely, which previously serialized the whole C->A
   transition (measured: layer period 19.7us -> 16.8us).
 - The energy sum uses sqrt(hp) precomputed on host: e = sum((re*rt)^2 +
   (im*rt)^2): 3 DVE passes per chunk with the reduction in the DVE
   accumulator (no activation table needed anywhere); the final chunk is
   split into two PSUM halves (pre/pim in separate banks!) to halve the
   closing serial chain, and each partial column streams out over its
   own small DMA as soon as it is ready.
 - All input DMAs issue from the otherwise-idle Sync queue (+gpsimd);
   the first A-group's operands lead the queues so the PE starts ASAP.
 - FINE-GRAINED PSUM RELEASE (worth ~6us): every phase's PSUM lives in
   single-bank tiles (separate pre/pim tiles for C; two half-tiles with
   per-half drains for each A/B group), because the tile framework's
   WAR tracking is per-tile -- a shared tile makes downstream matmuls
   wait for ALL of the tile's readers (drains/rotations), serializing
   the PE stream. With per-bank tiles each bank frees the moment its
   own consumer finishes. Same reason the L3 energy square-accumulates
   are deferred behind all the PSUM-freeing tt multiplies.
"""

import numpy as np

import concourse.bass as bass
import concourse.mybir as mybir
import concourse.tile as tile
from concourse import bacc
from concourse.bass_utils import run_bass_kernel_spmd

N = 18
DIM = 1 << N
P = 128
F = DIM // P  # 2048
LAYERS = 4
BATCH = 8
NCORES = 8

FP32 = mybir.dt.float32
FP16 = mybir.dt.float16
ALU = mybir.AluOpType
ACT = mybir.ActivationFunctionType

# ----------------------------------------------------------------------------
# Host-side math: hp diagonal, gate matrices, bit-layout permutations
# ----------------------------------------------------------------------------


def _compute_hp(adj):
    W = (np.triu(adj, k=1) > 0.5).astype(np.float64)
    n_edges = W.sum()
    idx = np.arange(DIM)
    shifts = (N - 1 - np.arange(N))[:, None]
    Z = 1.0 - 2.0 * ((idx[None, :] >> shifts) & 1).astype(np.float64)
    T = W @ Z
    cross = np.einsum("ud,ud->d", T, Z)
    return 0.5 * (n_edges - cross)  # [DIM], integer-valued*0.5, exact


def _rx(beta):
    c, s = np.cos(beta), np.sin(beta)
    return np.array([[c, -1j * s], [-1j * s, c]], dtype=np.complex128)


def _kron_list(mats):
    out = np.array([[1.0]], dtype=np.complex128)
    for m in mats:
        out = np.kron(out, m)
    return out


def _m7(beta):
    return _kron_list([_rx(beta)] * 7)


def _m41(beta):
    return _kron_list([_rx(beta)] * 4 + [np.eye(2, dtype=np.complex128)] * 3)


def _bitmap_after_A(bm):
    new = [0] * N
    for j in range(7):
        new[11 + j] = bm[j]
    for j in range(4):
        new[7 + j] = bm[7 + j]
    for j in range(7):
        new[j] = bm[11 + j]
    return new


def _bitmap_after_B(bm):
    # window = free bits 10..4 (single strided AP dim), tiles = bits 3..0
    new = [0] * N
    for j in range(7):
        new[11 + j] = bm[4 + j]
    for j in range(4):
        new[7 + j] = bm[j]
    for j in range(7):
        new[j] = bm[11 + j]
    return new


def _perm_for_bitmap(bm):
    a = np.arange(DIM, dtype=np.int64)
    out = np.zeros(DIM, dtype=np.int64)
    for j in range(N):
        out |= ((a >> j) & 1) << bm[j]
    return out


def _layer_perms():
    """Permutations (orig_idx = perm[cur_idx]) for the state layout at the
    start of each layer (1..LAYERS) plus the final layout (index LAYERS)."""
    perms = []
    bm = list(range(N))
    for _ in range(LAYERS):
        perms.append(_perm_for_bitmap(bm))
        bm = _bitmap_after_B(_bitmap_after_A(bm))
    perms.append(_perm_for_bitmap(bm))
    return perms


_PERMS = _layer_perms()


def _host_prep(batch_betas, adj_matrices):
    """Build per-core input dicts."""
    in_maps = []
    for b in range(BATCH):
        hp = _compute_hp(np.asarray(adj_matrices[b], dtype=np.float64))
        cos_hp = np.cos(hp)
        sin_hp = np.sin(hp)

        init_re = (
            cos_hp[_PERMS[0]].astype(np.float16).reshape(P, 4, 512)
            .transpose(1, 0, 2).copy()
        )
        init_im = (
            (-sin_hp[_PERMS[0]]).astype(np.float16).reshape(P, 4, 512)
            .transpose(1, 0, 2).copy()
        )

        diags = np.empty((2 * (LAYERS - 1) + 1, P, F), dtype=np.float16)
        for t in range(1, LAYERS):
            diags[2 * (t - 1)] = cos_hp[_PERMS[t]].astype(np.float16).reshape(P, F)
            diags[2 * (t - 1) + 1] = sin_hp[_PERMS[t]].astype(np.float16).reshape(P, F)
        # sqrt(hp): hp is a nonnegative multiple of 0.5, so this is exact
        # enough in fp16 and lets the energy be sum((s*rt)^2).
        diags[-1] = np.sqrt(hp)[_PERMS[LAYERS]].astype(np.float16).reshape(P, F)

        gates_ab = np.empty((LAYERS, P, 512), dtype=np.float16)
        gates_c = np.empty((LAYERS, P, 384), dtype=np.float16)
        for t in range(LAYERS):
            beta = float(np.asarray(batch_betas[b][t], dtype=np.float64))
            M7 = _m7(beta)
            C7 = M7.real.astype(np.float16)
            D7 = M7.imag.astype(np.float16)
            M41 = _m41(beta)
            C41 = M41.real.astype(np.float16)
            D41 = M41.imag.astype(np.float16)
            gates_ab[t, :, 0:128] = C7
            gates_ab[t, :, 128:256] = D7
            gates_ab[t, :, 256:384] = -D7
            gates_ab[t, :, 384:512] = C7
            gates_c[t, :, 0:128] = C41
            gates_c[t, :, 128:256] = -D41
            gates_c[t, :, 256:384] = D41

        in_maps.append(
            {
                "init_re": init_re,
                "init_im": init_im,
                "diags": diags,
                "gates_ab": gates_ab,
                "gates_c": gates_c,
            }
        )
    return in_maps


# ----------------------------------------------------------------------------
# Bass program
# ----------------------------------------------------------------------------


def _build_program():
    nc = bacc.Bacc("TRN2", target_bir_lowering=False, debug=False)

    d_init_re = nc.dram_tensor("init_re", [4, P, 512], FP16, kind="ExternalInput")
    d_init_im = nc.dram_tensor("init_im", [4, P, 512], FP16, kind="ExternalInput")
    d_diags = nc.dram_tensor(
        "diags", [2 * (LAYERS - 1) + 1, P, F], FP16, kind="ExternalInput"
    )
    d_gates_ab = nc.dram_tensor("gates_ab", [LAYERS, P, 512], FP16, kind="ExternalInput")
    d_gates_c = nc.dram_tensor("gates_c", [LAYERS, P, 384], FP16, kind="ExternalInput")
    d_out = nc.dram_tensor("out", [P, 8], FP32, kind="ExternalOutput")

    n_diag = 2 * (LAYERS - 1) + 1

    def stt(eng, out, in0, in1, op, scalar=1.0, accum_out=None):
        # (in0 * scalar) op in1 -- plain fp16 tensor_tensor is ~2x slower
        eng.scalar_tensor_tensor(out, in0, scalar, in1, ALU.mult, op,
                                 accum_out=accum_out)

    with tile.TileContext(nc) as tc:
        with (
            tc.tile_pool(name="state", bufs=1) as st_pool,
            tc.tile_pool(name="consts", bufs=1) as c_pool,
            tc.tile_pool(name="rot", bufs=3) as h_pool,
            tc.tile_pool(name="ps_mm", bufs=2, space="PSUM") as ps_pool,
            tc.tile_pool(name="ps_c", bufs=2, space="PSUM") as ps_c,
        ):
            # fused state tile: s_re = s2[:, 0:F], s_im = s2[:, F:2F]
            # (used only for the DMA'd initial state of layer 0)
            s2 = st_pool.tile([P, 2 * F], FP16, tag="s2")
            # Uncombined rotation products for layers 1..3: phase A's PSUM
            # accumulation performs the complex-combine adds for free:
            #   s_re = mRe0 + mRe1,  s_im = mIm0 + mIm1
            # so the rotation is only 4 DVE multiplies per chunk, no adds.
            mRe0 = st_pool.tile([P, F], FP16, tag="mRe0")
            mRe1 = st_pool.tile([P, F], FP16, tag="mRe1")
            mIm0 = st_pool.tile([P, F], FP16, tag="mIm0")
            mIm1 = st_pool.tile([P, F], FP16, tag="mIm1")
            # fused t1 tile: t1_re = [:, 0:F], t1_im = [:, F:2F]
            t1 = st_pool.tile([P, 2 * F], FP16, tag="t1")
            # B output keeps PSUM's interleaved layout (per 128-col window:
            # re|im) so each B drain is one contiguous copy; phase C reads
            # it through a strided moving AP.
            t2 = st_pool.tile([P, 2 * F], FP16, tag="t2")

            diag_t = [
                c_pool.tile([P, F], FP16, tag=f"diag{k}", name=f"diag{k}")
                for k in range(n_diag)
            ]
            gates_ab_t = [
                c_pool.tile([P, 512], FP16, tag=f"gab{t}", name=f"gab{t}")
                for t in range(LAYERS)
            ]
            gates_c_t = [
                c_pool.tile([P, 384], FP16, tag=f"gc{t}", name=f"gc{t}")
                for t in range(LAYERS)
            ]
            partial = c_pool.tile([P, 8], FP32, tag="partial")

            s_re = s2[:, 0:F]
            s_im = s2[:, F : 2 * F]
            # de-interleaved t1 views
            t1_re = t1[:, 0:F]
            t1_im = t1[:, F : 2 * F]

            # ---- PE warm-up: the PE reaches max clock only after ~3us of
            # continuous activity; dummy matmuls on a zeroed scratch tile
            # during the input-DMA wait let the real stream start at full
            # rate. The warm-up PSUM tile is never read.
            warm = c_pool.tile([P, 256], FP16, tag="warm")
            nc.vector.memset(warm[:], 0.0)
            wps = ps_pool.tile([P, 512], FP32, tag="ps0", name="warmps")
            for _ in range(12):
                nc.tensor.matmul(wps[:, 0:256], warm[:, 0:128], warm[:],
                                 start=True, stop=True)

            # ---- input DMAs: the three operands of the first A-group lead
            # three different queues so the first matmul starts ASAP.
            nc.sync.dma_start(gates_ab_t[0][:], d_gates_ab.ap()[0])
            nc.gpsimd.dma_start(s_re[:, 0:512], d_init_re.ap()[0])
            nc.sync.dma_start(s_im[:, 0:512], d_init_im.ap()[0])
            for c in range(1, 4):
                q = nc.sync if c % 2 == 1 else nc.gpsimd
                q.dma_start(s_re[:, 512 * c : 512 * (c + 1)], d_init_re.ap()[c])
                q2 = nc.gpsimd if c % 2 == 1 else nc.sync
                q2.dma_start(s_im[:, 512 * c : 512 * (c + 1)], d_init_im.ap()[c])
            nc.sync.dma_start(gates_c_t[0][:], d_gates_c.ap()[0])
            for t in range(1, LAYERS):
                nc.sync.dma_start(gates_ab_t[t][:], d_gates_ab.ap()[t])
                nc.sync.dma_start(gates_c_t[t][:], d_gates_c.ap()[t])

            def issue_diag_dmas(t):
                if t < LAYERS - 1:
                    nc.sync.dma_start(diag_t[2 * t][:], d_diags.ap()[2 * t])
                    nc.sync.dma_start(diag_t[2 * t + 1][:], d_diags.ap()[2 * t + 1])
                if t == LAYERS - 2:
                    # sqrt(hp) diagonal for the final energy reduction
                    nc.sync.dma_start(diag_t[n_diag - 1][:], d_diags.ap()[n_diag - 1])

            # strided views: B stationary windows / C moving chunks
            t1r4 = t1_re.rearrange("p (w u) -> p w u", w=128)
            t1i4 = t1_im.rearrange("p (w u) -> p w u", w=128)
            t2v = t2[:].rearrange("p (u ri j) -> p u ri j", u=16, ri=2)

            for t in range(LAYERS):
                cd7 = gates_ab_t[t][:, 0:256]
                ndc7 = gates_ab_t[t][:, 256:512]
                c41 = gates_c_t[t][:, 0:128]
                nd41 = gates_c_t[t][:, 128:256]
                d41 = gates_c_t[t][:, 256:384]

                # ---- phase A: stationary = state (contiguous 128-col windows).
                # Layer 0 reads the DMA'd state (2 matmuls/window); layers
                # 1..3 read the 4 uncombined rotation products (4 matmuls/
                # window) -- PSUM accumulation does the complex adds.
                dstv = t1[:].rearrange("p (ri g2 j T) -> p g2 j ri T", ri=2, g2=4, j=4)
                for g in range(4):
                    # two single-bank PSUM tiles per group: each half drains
                    # (and its bank frees) right after its own 2 windows
                    ph = [
                        ps_pool.tile([P, 512], FP32, tag=f"ps{h}", name=f"psA{t}{g}{h}")
                        for h in range(2)
                    ]
                    for j in range(4):
                        w = 4 * g + j
                        sl = slice(128 * w, 128 * (w + 1))
                        out_sl = ph[j // 2][:, 256 * (j % 2) : 256 * (j % 2 + 1)]
                        if t == 0:
                            nc.tensor.matmul(out_sl, s_re[:, sl], cd7, start=True, stop=False)
                            nc.tensor.matmul(out_sl, s_im[:, sl], ndc7, start=False, stop=True)
                        else:
                            # rotation chunks stay uncombined; PSUM adds them
                            nc.tensor.matmul(out_sl, mRe0[:, sl], cd7, start=True, stop=False)
                            nc.tensor.matmul(out_sl, mRe1[:, sl], cd7, start=False, stop=False)
                            nc.tensor.matmul(out_sl, mIm0[:, sl], ndc7, start=False, stop=False)
                            nc.tensor.matmul(out_sl, mIm1[:, sl], ndc7, start=False, stop=True)
                        if j % 2 == 1:
                            # de-interleaving half-drain: PSUM [j2; ri; T]
                            # scatters into the two halves of t1. The very
                            # last half (gating phase B) goes to the idle
                            # DVE so it runs parallel to Act's previous one.
                            h = j // 2
                            src = ph[h][:].rearrange("p (j2 ri T) -> p j2 ri T", j2=2, ri=2)
                            if g == 3 and h == 1:
                                nc.vector.tensor_copy(dstv[:, g, 2:4], src)
                            else:
                                nc.scalar.copy(dstv[:, g, 2 * h : 2 * h + 2], src)
                    if g == 0:
                        issue_diag_dmas(t)

                # ---- phases B and C, interleaved: B0 B1 C0 B2 C1 B3 C2 C3.
                # C chunk k only needs B group k's drain, so phase C (and the
                # rotation it feeds) starts mid-B, spreading the DVE rotation
                # work over a ~2x longer window and relieving PSUM pressure.
                def b_group(g):
                    ph = [
                        ps_pool.tile([P, 512], FP32, tag=f"ps{h}", name=f"psB{t}{g}{h}")
                        for h in range(2)
                    ]
                    for j in range(4):
                        u = 4 * g + j
                        out_sl = ph[j // 2][:, 256 * (j % 2) : 256 * (j % 2 + 1)]
                        nc.tensor.matmul(
                            out_sl, t1r4[:, :, u], cd7, start=True, stop=False
                        )
                        nc.tensor.matmul(
                            out_sl, t1i4[:, :, u], ndc7, start=False, stop=True
                        )
                        if j % 2 == 1:
                            h = j // 2
                            dst = slice(1024 * g + 512 * h, 1024 * g + 512 * (h + 1))
                            nc.scalar.copy(t2[:, dst], ph[h][:])

                def c_chunk(k):
                    if t == LAYERS - 1 and k == 3:
                        # final chunk: two independent PSUM half-chunks with
                        # their OWN tiles (sharing a tile would make half-b's
                        # matmuls wait on half-a's energy reads) so the
                        # closing energy chain is half as long
                        rt = diag_t[n_diag - 1]
                        for h in range(2):
                            pcr = ps_c.tile([P, 512], FP32, tag="pcr", name=f"pcr3{h}")
                            pci = ps_c.tile([P, 512], FP32, tag="pci", name=f"pci3{h}")
                            mv_re = t2v[:, 4 * k + 2 * h : 4 * k + 2 * h + 2, 0, :]
                            mv_im = t2v[:, 4 * k + 2 * h : 4 * k + 2 * h + 2, 1, :]
                            pre = pcr[:, 0:256]
                            pim = pci[:, 0:256]
                            nc.tensor.matmul(pre, c41, mv_re, start=True, stop=False)
                            nc.tensor.matmul(pim, c41, mv_im, start=True, stop=False)
                            nc.tensor.matmul(pre, nd41, mv_im, start=False, stop=True)
                            nc.tensor.matmul(pim, d41, mv_re, start=False, stop=True)
                            hk = slice(512 * k + 256 * h, 512 * k + 256 * (h + 1))
                            tt = h_pool.tile([P, 512], FP16, tag="th", name=f"tt3{h}")
                            sq = h_pool.tile([P, 512], FP16, tag="sh", name=f"sq3{h}")
                            stt(nc.vector, tt[:, 0:256], pre, rt[:, hk], ALU.mult)
                            stt(nc.vector, tt[:, 256:512], pim, rt[:, hk], ALU.mult)
                            stt(nc.vector, sq[:], tt[:], tt[:], ALU.mult,
                                accum_out=partial[:, 3 + h : 4 + h])
                            nc.sync.dma_start(d_out.ap()[:, 3 + h : 4 + h],
                                              partial[:, 3 + h : 4 + h])
                        return
                    # separate single-bank tiles for pre/pim so each bank is
                    # released as soon as its own rotation reads finish
                    pcr = ps_c.tile([P, 512], FP32, tag="pcr", name=f"pcr{t}{k}")
                    pci = ps_c.tile([P, 512], FP32, tag="pci", name=f"pci{t}{k}")
                    mv_re = t2v[:, 4 * k : 4 * (k + 1), 0, :]
                    mv_im = t2v[:, 4 * k : 4 * (k + 1), 1, :]
                    pre = pcr[:]
                    pim = pci[:]
                    nc.tensor.matmul(pre, c41, mv_re, start=True, stop=False)
                    nc.tensor.matmul(pim, c41, mv_im, start=True, stop=False)
                    nc.tensor.matmul(pre, nd41, mv_im, start=False, stop=True)
                    nc.tensor.matmul(pim, d41, mv_re, start=False, stop=True)

                    ck = slice(512 * k, 512 * (k + 1))
                    if t < LAYERS - 1:
                        # state = psC * exp(-i hp):
                        #   re' = re*c + im*s ; im' = im*c - re*s
                        # Only the 4 multiplies run here (DVE, reading PSUM
                        # directly; pre-consumers ordered first so its bank
                        # frees early); the combine adds happen inside next
                        # layer's phase-A PSUM accumulation via the pieces.
                        cs = diag_t[2 * t][:, ck]
                        sn = diag_t[2 * t + 1][:, ck]
                        stt(nc.vector, mRe0[:, ck], pre, cs, ALU.mult)
                        stt(nc.vector, mIm1[:, ck], pre, sn, ALU.mult,
                            scalar=-1.0)
                        stt(nc.vector, mRe1[:, ck], pim, sn, ALU.mult)
                        stt(nc.vector, mIm0[:, ck], pim, cs, ALU.mult)
                    else:
                        # energy: sum(|state*rt|^2), rt = sqrt(hp).
                        # Only the PSUM-freeing tt multiplies run here; the
                        # square-accumulates are deferred so later C chunks
                        # never wait on PSUM banks held by the backlog.
                        rt = diag_t[n_diag - 1][:, ck]
                        tt = h_pool.tile([P, 1024], FP16, tag="mm", name=f"tt{k}")
                        stt(nc.vector, tt[:, 0:512], pre, rt, ALU.mult)
                        stt(nc.vector, tt[:, 512:1024], pim, rt, ALU.mult)
                        pending_sq.append((tt, k))

                def flush_sq():
                    for tt, k in pending_sq:
                        sq = h_pool.tile([P, 1024], FP16, tag="mn", name=f"sq{k}")
                        stt(nc.vector, sq[:], tt[:], tt[:], ALU.mult,
                            accum_out=partial[:, k : k + 1])
                        nc.sync.dma_start(d_out.ap()[:, k : k + 1],
                                          partial[:, k : k + 1])
                    pending_sq.clear()

                pending_sq = []
                b_group(0)
                b_group(1)
                c_chunk(0)
                b_group(2)
                c_chunk(1)
                b_group(3)
                c_chunk(2)
                flush_sq()
                c_chunk(3)

    nc.compile()
    return nc


_NC_CACHE = {}


def _get_program():
    if "nc" not in _NC_CACHE:
        _NC_CACHE["nc"] = _build_program()
    return _NC_CACHE["nc"]


def kernel(batch_betas, adj_matrices, _trace=False, _tmpdir=None):
    batch_betas = np.asarray(batch_betas, dtype=np.float32)
    adj_matrices = np.asarray(adj_matrices, dtype=np.float32)
    assert batch_betas.shape == (BATCH, LAYERS)
    assert adj_matrices.shape == (BATCH, N, N)

    nc = _get_program()
    in_maps = _host_prep(batch_betas, adj_matrices)
    res = run_bass_kernel_spmd(
        nc,
        in_maps,
        list(range(NCORES)),
        trace=_trace,
        tmpdir=_tmpdir,
    )
    energies = np.array(
        [res.results[b]["out"][:, 0:5].sum() / DIM for b in range(BATCH)],
        dtype=np.float32,
    )
    if _trace:
        return energies, res
    return energies
